# revision 21
# baseline (speedup 1.0000x reference)
"""Trainium2 Bass kernel for nn_MultiHeadAttention_31052613550603.

Sharding: tensor-parallel over heads. 16 heads / 8 cores = 2 heads per core.
Each core computes Q/K/V projections for its 2 heads, full (non-causal)
softmax attention, and its row-shard of the output projection Wo. The
all-reduce of the 8 partial outputs is done host-side (cheap numpy sum).

Per-core device layout (batch processed sequentially, b=0,1):
  xT        [D=1024, S=2048] fp16  (host-transposed embeddings)
  QT2,KT2   [128, 2048] fp16       rows 0:64 head0's Q^T/K^T, 64:128 head1's
  V2T       [128, 2048] fp16       V^T, transposed on-device (XBAR DMA) to
  v2n       [128, 16, 128] fp16    V in natural [k,d] layout, per k-tile
  scores    S^T[k,q] in PSUM fp32, exp on ACT -> PT fp16 [k,q]
  O^T       accumulated in PSUM over k-tiles:  O^T[d,q] += V^T P^T
  Z         column sums of PT via ones-vector matmul on accumulated PTsum
  out       O^T/Z (2 heads stacked = 128 rows) @ Wo[128 rows] -> partial out
"""

import os
import numpy as np

import concourse.bass as bass
import concourse.tile as tile
from concourse import bacc, mybir
from concourse.bass import ts
from concourse.bass_utils import run_bass_kernel_spmd

F16 = mybir.dt.float16
F32 = mybir.dt.float32
F32R = mybir.dt.float32r
EXP = mybir.ActivationFunctionType.Exp

B, S, D, H, DK, DV = 2, 2048, 1024, 16, 64, 64
NCORES = 8
HPC = H // NCORES          # heads per core = 2
D2 = HPC * DV              # 128, stacked head dim
QB = 512                   # q block (columns per attention pass)
NQB = S // QB              # 4
NKT = S // 128             # 16 k-tiles
MCH = D // 128             # 8 m-chunks for projections
NO_B = D // 512            # 2 n-blocks of output projection
SCALE = 1.0 / float(np.sqrt(DK))


def build(debug: bool = False, nrep: int = 1, abl: int = 0):
    nc = bacc.Bacc("TRN2", target_bir_lowering=False, debug=debug,
                   num_devices=NCORES)

    xT = nc.dram_tensor("xT", [B, D, S], F16, kind="ExternalInput").ap()
    wq2 = nc.dram_tensor("wq2", [D, D2], F16, kind="ExternalInput").ap()
    wk2 = nc.dram_tensor("wk2", [D, D2], F16, kind="ExternalInput").ap()
    wv2 = nc.dram_tensor("wv2", [D, D2], F16, kind="ExternalInput").ap()
    bq2 = nc.dram_tensor("bq2", [D2, 1], F32, kind="ExternalInput").ap()
    bk2 = nc.dram_tensor("bk2", [D2, 1], F32, kind="ExternalInput").ap()
    bv2 = nc.dram_tensor("bv2", [D2, 1], F32, kind="ExternalInput").ap()
    wo2 = nc.dram_tensor("wo2", [D2, D], F16, kind="ExternalInput").ap()
    ones1 = nc.dram_tensor("ones1", [128, 1], F16, kind="ExternalInput").ap()
    e2 = nc.dram_tensor("e2", [33, 128], F16, kind="ExternalInput").ap()
    out_p = nc.dram_tensor("out_p", [B * S, D], F16, kind="ExternalOutput").ap()

    from contextlib import ExitStack
    with tile.TileContext(nc) as tc, ExitStack() as ctx:
        consts = ctx.enter_context(tc.tile_pool(name="consts", bufs=1))
        xt_pool = ctx.enter_context(tc.tile_pool(name="xt", bufs=2))
        proj_pool = ctx.enter_context(tc.tile_pool(name="projT", bufs=2))
        v2n_pool = ctx.enter_context(tc.tile_pool(name="v2n", bufs=2))
        ot_pool = ctx.enter_context(tc.tile_pool(name="ot", bufs=2))
        pt_pool = ctx.enter_context(tc.tile_pool(name="pt", bufs=6))
        pts_pool = ctx.enter_context(tc.tile_pool(name="pts", bufs=4))
        z2_pool = ctx.enter_context(tc.tile_pool(name="z2", bufs=2))
        osb_pool = ctx.enter_context(tc.tile_pool(name="osb", bufs=3))
        # PSUM pools.  8 banks: st2 tiles are 2 banks each (bufs=2 -> 4),
        # psO 1 bank (bufs=2), shared [128,512] pool for proj/Zb/outproj.
        ps_st = ctx.enter_context(tc.tile_pool(name="ps_st", bufs=2, space="PSUM"))
        ps_o = ctx.enter_context(tc.tile_pool(name="ps_o", bufs=1, space="PSUM"))
        ps_mm = ctx.enter_context(tc.tile_pool(name="ps_mm", bufs=2, space="PSUM"))

        # ---- constants ----
        w_sb = {}
        for name, ap in (("q", wq2), ("k", wk2), ("v", wv2)):
            t = consts.tile([128, MCH, D2], F16, name=f"w_{name}")
            nc.sync.dma_start(t[:], ap.rearrange("(mo p) c -> p mo c", p=128))
            w_sb[name] = t
        b_sb = {}
        for name, ap in (("q", bq2), ("k", bk2), ("v", bv2)):
            t = consts.tile([D2, 1], F32, name=f"b_{name}")
            nc.sync.dma_start(t[:], ap)
            b_sb[name] = t
        wo_sb = consts.tile([D2, D], F16, name="wo")
        nc.sync.dma_start(wo_sb[:], wo2)
        ones_sb = consts.tile([128, 1], F16, name="ones")
        nc.sync.dma_start(ones_sb[:], ones1)
        e2_sb = consts.tile([33, 128], F16, name="e2")
        nc.sync.dma_start(e2_sb[:], e2)

        from contextlib import nullcontext
        rep_ctx = (tc.For_i(0, nrep, 1,
                            hint_engines=(mybir.EngineType.PE,
                                          mybir.EngineType.DVE,
                                          mybir.EngineType.Activation,
                                          mybir.EngineType.SP))
                   if nrep > 1 else nullcontext())
        with rep_ctx:
          for b in range(B):
            # ---- load x^T for this batch ----
            xt_sb = xt_pool.tile([128, MCH, S], F16, tag="xt")
            for m in range(MCH):
                nc.sync.dma_start(xt_sb[:, m, :], xT[b, m * 128:(m + 1) * 128, :])

            # ---- projections: K first, then V, then Q (attention q_blk 0
            # needs all of K,V but only the first q-block of Q) ----
            projT = {}
            for name in ("k", "v", "q"):
                dst = proj_pool.tile([D2, S], F16, tag=f"projT_{name}")
                projT[name] = dst
                for j in range(NQB):
                    ps = ps_mm.tile([128, 512], F32, tag="mm")
                    for m in range(MCH):
                        nc.tensor.matmul(ps[:], lhsT=w_sb[name][:, m, :],
                                         rhs=xt_sb[:, m, ts(j, 512)],
                                         start=(m == 0), stop=(m == MCH - 1))
                    nc.vector.tensor_scalar(dst[:, ts(j, 512)], ps[:],
                                            b_sb[name][:], None,
                                            op0=mybir.AluOpType.add)

            # ---- V -> natural layout via XBAR transpose, then repack to
            # [V_h | ones] 66-col groups so PV accumulates Z in psum row 64
            v2n = v2n_pool.tile([128, NKT, D2], F16, tag="v2n")
            for t in range(NKT):
                nc.sync.dma_start_transpose(v2n[:, t, :], projT["v"][:, ts(t, 128)])
            v2np = v2n_pool.tile([128, NKT, 132], F16, tag="v2np")
            ones_view = v2np[:].rearrange("p t (h c) -> p t h c", h=2)[:, :, :, 64:65]
            nc.gpsimd.memset(ones_view, 1.0)
            for t in range(NKT):
                nc.gpsimd.tensor_copy(
                    v2np[:, t].rearrange("p (h c) -> p h c", h=2)[:, :, 0:64],
                    v2n[:, t].rearrange("p (h c) -> p h c", h=2))

            # ---- attention ----
            ot_sb = ot_pool.tile([D2, S], F16, tag="ot")
            if abl >= 2:
                nc.vector.memset(ot_sb[:], 0.01)
            for j in range(NQB):
                psO0 = ps_o.tile([65, QB], F32, tag="psO0")
                psO1 = ps_o.tile([65, QB], F32, tag="psO1")
                for t in range(NKT):
                    if abl >= 4:
                        continue
                    st2 = ps_st.tile([128, 2 * QB], F32, tag="st")
                    nc.tensor.matmul(st2[:, 0:QB],
                                     lhsT=projT["k"][0:64, ts(t, 128)],
                                     rhs=projT["q"][0:64, ts(j, QB)],
                                     start=True, stop=True)
                    nc.tensor.matmul(st2[:, QB:2 * QB],
                                     lhsT=projT["k"][64:128, ts(t, 128)],
                                     rhs=projT["q"][64:128, ts(j, QB)],
                                     start=True, stop=True)
                    if abl >= 3:
                        continue
                    pt2 = pt_pool.tile([128, 2 * QB], F16, tag="pt")
                    nc.scalar.activation(pt2[:], st2[:], EXP, scale=SCALE)
                    if abl >= 2:
                        continue
                    nc.tensor.matmul(psO0[:, :], lhsT=v2np[:, t, 0:65],
                                     rhs=pt2[:, 0:QB],
                                     start=(t == 0), stop=(t == NKT - 1))
                    nc.tensor.matmul(psO1[:, :], lhsT=v2np[:, t, 66:131],
                                     rhs=pt2[:, QB:2 * QB],
                                     start=(t == 0), stop=(t == NKT - 1))

                if abl == 1:
                    nc.vector.tensor_copy(ot_sb[0:64, ts(j, QB)], psO0[0:64, :])
                    nc.vector.tensor_copy(ot_sb[64:128, ts(j, QB)], psO1[0:64, :])
                if abl >= 1:
                    for i in range(4 * j, 4 * j + 4):
                        for nb in range(NO_B):
                            pso = ps_mm.tile([128, 512], F32, tag="mm")
                            nc.tensor.matmul(pso[:], lhsT=ot_sb[:, ts(i, 128)],
                                             rhs=wo_sb[:, ts(nb, 512)],
                                             start=True, stop=True)
                            osb = osb_pool.tile([128, 512], F16, tag="osb")
                            nc.vector.tensor_copy(osb[:], pso[:])
                            nc.sync.dma_start(
                                out_p[b * S + i * 128: b * S + (i + 1) * 128,
                                      ts(nb, 512)], osb[:])
                    continue
                z2 = z2_pool.tile([33, QB], F16, tag="z2")
                with nc.allow_low_precision(reason="1/Z in fp16; adds ~2e-4 rel"):
                    nc.vector.reciprocal(z2[0:1, :], psO0[64:65, :])
                    nc.vector.reciprocal(z2[32:33, :], psO1[64:65, :])
                psZb = ps_mm.tile([128, 512], F32, tag="mm")
                nc.tensor.matmul(psZb[:], lhsT=e2_sb[0:1, :],
                                 rhs=z2[0:1, :], start=True, stop=False,
                                 skip_group_check=True)
                nc.tensor.matmul(psZb[:], lhsT=e2_sb[32:33, :],
                                 rhs=z2[32:33, :], start=False, stop=True,
                                 skip_group_check=True)
                zb_sb = z2_pool.tile([128, QB], F32, tag="zb")
                nc.vector.tensor_copy(zb_sb[:], psZb[:])
                nc.vector.tensor_mul(ot_sb[0:64, ts(j, QB)], psO0[0:64, :],
                                     zb_sb[0:64, :])
                nc.vector.tensor_mul(ot_sb[64:128, ts(j, QB)], psO1[0:64, :],
                                     zb_sb[64:128, :])

                # ---- output projection for the 4 s-tiles of this q block ----
                for i in range(4 * j, 4 * j + 4):
                    for nb in range(NO_B):
                        pso = ps_mm.tile([128, 512], F32, tag="mm")
                        nc.tensor.matmul(pso[:], lhsT=ot_sb[:, ts(i, 128)],
                                         rhs=wo_sb[:, ts(nb, 512)],
                                         start=True, stop=True)
                        osb = osb_pool.tile([128, 512], F16, tag="osb")
                        nc.vector.tensor_copy(osb[:], pso[:])
                        nc.sync.dma_start(
                            out_p[b * S + i * 128: b * S + (i + 1) * 128,
                                  ts(nb, 512)], osb[:])

    nc.compile()
    return nc


_NC_CACHE = {}


def _get_nc():
    if "nc" not in _NC_CACHE:
        _NC_CACHE["nc"] = build()
    return _NC_CACHE["nc"]


def make_in_maps(embeddings, Wq, bq, Wk, bk, Wv, bv, Wo, bo):
    embeddings = np.asarray(embeddings, dtype=np.float32)
    Wq, Wk, Wv = (np.asarray(a, np.float32) for a in (Wq, Wk, Wv))
    bq, bk, bv = (np.asarray(a, np.float32) for a in (bq, bk, bv))
    Wo = np.asarray(Wo, np.float32)

    xT = np.ascontiguousarray(embeddings.transpose(0, 2, 1)).astype(np.float16)
    ones1 = np.ones((128, 1), np.float16)
    e2 = np.zeros((33, 128), np.float16)
    e2[0, 0:64] = 1.0
    e2[32, 64:128] = 1.0

    in_maps = []
    for c in range(NCORES):
        h0, h1 = HPC * c, HPC * c + 1
        in_maps.append({
            "xT": xT,
            "wq2": np.concatenate([Wq[h0], Wq[h1]], axis=1).astype(np.float16),
            "wk2": np.concatenate([Wk[h0], Wk[h1]], axis=1).astype(np.float16),
            "wv2": np.concatenate([Wv[h0], Wv[h1]], axis=1).astype(np.float16),
            "bq2": np.concatenate([bq[h0], bq[h1]])[:, None].astype(np.float32),
            "bk2": np.concatenate([bk[h0], bk[h1]])[:, None].astype(np.float32),
            "bv2": np.concatenate([bv[h0], bv[h1]])[:, None].astype(np.float32),
            "wo2": Wo[HPC * DV * c: HPC * DV * (c + 1), :].astype(np.float16),
            "ones1": ones1,
            "e2": e2,
        })
    return in_maps


def kernel(embeddings, Wq, bq, Wk, bk, Wv, bv, Wo, bo):
    nc = _get_nc()
    in_maps = make_in_maps(embeddings, Wq, bq, Wk, bk, Wv, bv, Wo, bo)
    res = run_bass_kernel_spmd(nc, in_maps, core_ids=list(range(NCORES)))
    acc = np.zeros((B * S, D), np.float32)
    for r in res.results:
        acc += r["out_p"].astype(np.float32)
    acc += np.asarray(bo, np.float32)[None, :]
    return acc.reshape(B, S, D)


# revision 37
# speedup vs baseline: 1.0187x; 1.0187x over previous
"""Trainium2 Bass kernel for nn_MultiHeadAttention_31052613550603.

Sharding: tensor-parallel over heads. 16 heads / 8 cores = 2 heads per core.
Each core computes Q/K/V projections for its 2 heads, full (non-causal)
softmax attention, and its row-shard of the output projection Wo. The
all-reduce of the 8 partial outputs is done host-side (cheap numpy sum).

Per-core pipeline (fp16 operands, fp32 PSUM):
  xT      [D, S] fp16 host-transposed embeddings
  QT/KT   [128, S]: rows 0:64 head0's Q^T/K^T, rows 64:128 head1's
  V^T -> XBAR-transposed to v2np [128, NKT, 2*(64+4+4)] = [V_h | 1111 pad]
  scores  S^T[k,q] fp32 in PSUM (both heads in one 2-bank tile),
          exp on ACT -> PT fp16; PV matmuls accumulate O^T (and the softmax
          denominator Z via the four ones-columns -> psum rows 64:68)
  norm    1/Z broadcast via tiny 2-row matmul, O^T scaled on DVE
  outproj OT[128, s-tile] stationary x Wo rows -> partial out, fp16 to HBM

Emission order software-pipelines everything: batch 1's projection groups are
interleaved into batch 0's attention so PE never starves ACT, and PV matmuls
lag ST/exp by `pv_lag` k-tiles so the per-q-block normalization chain hides.
"""

import numpy as np

import concourse.bass as bass
import concourse.tile as tile
from concourse import bacc, mybir
from concourse.bass import ts
from concourse.bass_utils import run_bass_kernel_spmd

F16 = mybir.dt.float16
F32 = mybir.dt.float32
EXP = mybir.ActivationFunctionType.Exp

B, S, D, H, DK, DV = 2, 2048, 1024, 16, 64, 64
NCORES = 8
HPC = H // NCORES          # heads per core = 2
D2 = HPC * DV              # 128
QB = 512                   # q block
NQB = S // QB              # 4
NKT = S // 128             # 16 k-tiles
MCH = D // 128             # 8 m-chunks
NO_B = D // 512            # 2 out-proj n-blocks
VW = 128                   # per-head stride in v2np: 64 V + 64 ones
SCALE = 1.0 / float(np.sqrt(DK))


def build(debug: bool = False, nrep: int = 1, pv_lag: int = 6):
    nc = bacc.Bacc("TRN2", target_bir_lowering=False, debug=debug,
                   num_devices=NCORES)

    xT = nc.dram_tensor("xT", [B, D, S], F16, kind="ExternalInput").ap()
    wq2 = nc.dram_tensor("wq2", [D, D2], F16, kind="ExternalInput").ap()
    wk2 = nc.dram_tensor("wk2", [D, D2], F16, kind="ExternalInput").ap()
    wv2 = nc.dram_tensor("wv2", [D, D2], F16, kind="ExternalInput").ap()
    bq2 = nc.dram_tensor("bq2", [D2, 1], F32, kind="ExternalInput").ap()
    bk2 = nc.dram_tensor("bk2", [D2, 1], F32, kind="ExternalInput").ap()
    bv2 = nc.dram_tensor("bv2", [D2, 1], F32, kind="ExternalInput").ap()
    wo2 = nc.dram_tensor("wo2", [D2, D], F16, kind="ExternalInput").ap()
    e2 = nc.dram_tensor("e2", [33, 128], F16, kind="ExternalInput").ap()
    out_t = nc.dram_tensor("out_t", [B * NQB, 128, 4, D], F16,
                           kind="ExternalOutput").ap()

    from contextlib import ExitStack, nullcontext
    with tile.TileContext(nc) as tc, ExitStack() as ctx:
        consts = ctx.enter_context(tc.tile_pool(name="consts", bufs=1))
        xt_pool = ctx.enter_context(tc.tile_pool(name="xt", bufs=2))
        proj_pool = ctx.enter_context(tc.tile_pool(name="projT", bufs=2))
        v2n_pool = ctx.enter_context(tc.tile_pool(name="v2n", bufs=2))
        ot_pool = ctx.enter_context(tc.tile_pool(name="ot", bufs=2))
        pt_pool = ctx.enter_context(tc.tile_pool(name="pt", bufs=pv_lag + 3))
        z2_pool = ctx.enter_context(tc.tile_pool(name="z2", bufs=2))
        osb_pool = ctx.enter_context(tc.tile_pool(name="osb", bufs=2))
        ps_st = ctx.enter_context(tc.tile_pool(name="ps_st", bufs=2, space="PSUM"))
        ps_o = ctx.enter_context(tc.tile_pool(name="ps_o", bufs=1, space="PSUM"))
        ps_mm = ctx.enter_context(tc.tile_pool(name="ps_mm", bufs=2, space="PSUM"))

        # ---- constants ----
        w_sb = {}
        for name, ap in (("q", wq2), ("k", wk2), ("v", wv2)):
            t = consts.tile([128, MCH, D2], F16, name=f"w_{name}")
            nc.sync.dma_start(t[:], ap.rearrange("(mo p) c -> p mo c", p=128))
            w_sb[name] = t
        b_sb = {}
        for name, ap in (("q", bq2), ("k", bk2), ("v", bv2)):
            t = consts.tile([D2, 1], F32, name=f"b_{name}")
            nc.sync.dma_start(t[:], ap)
            b_sb[name] = t
        wo_sb = consts.tile([D2, D], F16, name="wo")
        nc.sync.dma_start(wo_sb[:], wo2)
        e2_sb = consts.tile([33, 128], F16, name="e2")
        nc.sync.dma_start(e2_sb[:], e2)

        rep_ctx = (tc.For_i(0, nrep, 1,
                            hint_engines=(mybir.EngineType.PE,
                                          mybir.EngineType.DVE,
                                          mybir.EngineType.Activation,
                                          mybir.EngineType.SP))
                   if nrep > 1 else nullcontext())
        with rep_ctx:
            # per-batch state, filled in by the emitters below
            st = [dict() for _ in range(B)]

            def emit_xt_load(b):
                xt_sb = xt_pool.tile([128, MCH, S], F16, tag="xt",
                                     name=f"xt_{b}")
                st[b]["xt"] = xt_sb
                for m in range(MCH):
                    nc.sync.dma_start(xt_sb[:, m, :],
                                      xT[b, m * 128:(m + 1) * 128, :])

            def emit_proj_alloc(b):
                for name in ("k", "v", "q"):
                    st[b][name] = proj_pool.tile([D2, S], F16,
                                                 tag=f"projT_{name}",
                                                 name=f"projT_{name}_{b}")

            def emit_proj_group(b, name, j):
                """One s-block of one projection: 8 accumulating matmuls."""
                dst = st[b][name]
                ps = ps_mm.tile([128, 512], F32, tag="mm")
                for m in range(MCH):
                    nc.tensor.matmul(ps[:], lhsT=w_sb[name][:, m, :],
                                     rhs=st[b]["xt"][:, m, ts(j, 512)],
                                     start=(m == 0), stop=(m == MCH - 1))
                nc.vector.tensor_scalar(dst[:, ts(j, 512)], ps[:],
                                        b_sb[name][:], None,
                                        op0=mybir.AluOpType.add)

            def emit_proj_mouter(b, name, js):
                """Two s-blocks of one projection, m-outer so the first
                matmuls start as soon as xt chunk 0 lands (short lead-in)."""
                dst = st[b][name]
                pss = [ps_mm.tile([128, 512], F32, tag="mm",
                                  name=f"mo_{name}_{b}_{jj}") for jj in js]
                for m in range(MCH):
                    for ps, jj in zip(pss, js):
                        nc.tensor.matmul(ps[:], lhsT=w_sb[name][:, m, :],
                                         rhs=st[b]["xt"][:, m, ts(jj, 512)],
                                         start=(m == 0), stop=(m == MCH - 1))
                for ps, jj in zip(pss, js):
                    nc.vector.tensor_scalar(dst[:, ts(jj, 512)], ps[:],
                                            b_sb[name][:], None,
                                            op0=mybir.AluOpType.add)

            def emit_v_transpose(b):
                v2np = v2n_pool.tile([128, NKT, 2 * VW], F16, tag="v2np",
                                     name=f"v2np_{b}")
                st[b]["v2np"] = v2np
                ones_view = v2np[:].rearrange("p t (h c) -> p t h c",
                                              h=2)[:, :, :, 64:VW]
                nc.vector.memset(ones_view, 1.0)
                nc.scalar.dma_start_transpose(v2np[:, :, 0:64],
                                              st[b]["v"][0:64, :])
                nc.scalar.dma_start_transpose(v2np[:, :, VW:VW + 64],
                                              st[b]["v"][64:128, :])

            def emit_ot_alloc(b):
                st[b]["ot"] = ot_pool.tile([D2, S], F16, tag="ot",
                                            name=f"ot_{b}")

            def emit_attention_qblk(b, j, filler):
                """ST/exp/PV pipeline for one q block; `filler()` is called
                once per k-tile to interleave lower-priority PE work."""
                K2, Q2 = st[b]["k"], st[b]["q"]
                v2np = st[b]["v2np"]
                psO0 = ps_o.tile([128, QB], F32, tag="psO0")
                psO1 = ps_o.tile([128, QB], F32, tag="psO1")
                pending = []

                def emit_pv(tt, ptt):
                    nc.tensor.matmul(psO0[:, :], lhsT=v2np[:, tt, 0:VW],
                                     rhs=ptt[:, 0:QB],
                                     start=(tt == 0), stop=(tt == NKT - 1))
                    nc.tensor.matmul(psO1[:, :], lhsT=v2np[:, tt, VW:2 * VW],
                                     rhs=ptt[:, QB:2 * QB],
                                     start=(tt == 0), stop=(tt == NKT - 1))

                for t in range(NKT):
                    st2 = ps_st.tile([128, 2 * QB], F32, tag="st")
                    nc.tensor.matmul(st2[:, 0:QB],
                                     lhsT=K2[0:64, ts(t, 128)],
                                     rhs=Q2[0:64, ts(j, QB)],
                                     start=True, stop=True)
                    nc.tensor.matmul(st2[:, QB:2 * QB],
                                     lhsT=K2[64:128, ts(t, 128)],
                                     rhs=Q2[64:128, ts(j, QB)],
                                     start=True, stop=True)
                    pt2 = pt_pool.tile([128, 2 * QB], F16, tag="pt")
                    nc.scalar.activation(pt2[:], st2[:], EXP, scale=SCALE)
                    pending.append((t, pt2))
                    if len(pending) > pv_lag:
                        emit_pv(*pending.pop(0))
                    filler()
                for _pv in pending:
                    emit_pv(*_pv)

                # normalization: Z sits in psO rows 64:68 (ones columns)
                z2 = z2_pool.tile([33, QB], F16, tag="z2")
                with nc.allow_low_precision(reason="1/Z fp16 ~2e-4 rel"):
                    nc.vector.reciprocal(z2[0:1, :], psO0[64:65, :])
                    nc.vector.reciprocal(z2[32:33, :], psO1[64:65, :])
                psZb = ps_mm.tile([128, 512], F32, tag="mm")
                nc.tensor.matmul(psZb[:], lhsT=e2_sb[0:1, :], rhs=z2[0:1, :],
                                 start=True, stop=False, skip_group_check=True)
                nc.tensor.matmul(psZb[:], lhsT=e2_sb[32:33, :],
                                 rhs=z2[32:33, :], start=False, stop=True,
                                 skip_group_check=True)
                zb_sb = z2_pool.tile([128, QB], F32, tag="zb")
                nc.vector.tensor_copy(zb_sb[:], psZb[:])
                ot = st[b]["ot"]
                nc.vector.tensor_mul(ot[0:64, ts(j, QB)], psO0[0:64, :],
                                     zb_sb[0:64, :])
                nc.vector.tensor_mul(ot[64:128, ts(j, QB)], psO1[0:64, :],
                                     zb_sb[64:128, :])

            def emit_outproj_qblk(b, j):
                ot = st[b]["ot"]
                osb4 = osb_pool.tile([128, 4, D], F16, tag="osb4")
                for i4 in range(4):
                    i = 4 * j + i4
                    for nb in range(NO_B):
                        pso = ps_mm.tile([128, 512], F32, tag="mm")
                        nc.tensor.matmul(pso[:], lhsT=ot[:, ts(i, 128)],
                                         rhs=wo_sb[:, ts(nb, 512)],
                                         start=True, stop=True)
                        nc.vector.tensor_copy(osb4[:, i4, ts(nb, 512)], pso[:])
                nc.sync.dma_start(out_t[b * NQB + j], osb4[:])

            # ---------- master schedule ----------
            emit_xt_load(0)
            emit_proj_alloc(0)
            emit_ot_alloc(0)
            emit_proj_mouter(0, "k", (0, 1))
            emit_proj_mouter(0, "k", (2, 3))
            emit_proj_group(0, "q", 0)
            emit_proj_mouter(0, "v", (0, 1))
            emit_proj_mouter(0, "v", (2, 3))
            emit_v_transpose(0)

            # deferred work, drained one item per ~3 attention k-tiles
            work = []
            for j in range(1, NQB):
                work.append(("g", 0, "q", j))
            work.append(("x", 1))
            work.append(("a", 1))
            for j in range(NQB):
                work.append(("g", 1, "k", j))
            for j in range(NQB):
                work.append(("g", 1, "v", j))
            work.append(("t", 1))
            for j in range(NQB):
                work.append(("g", 1, "q", j))

            def do_item(item):
                if item[0] == "g":
                    _, b, name, j = item
                    emit_proj_group(b, name, j)
                elif item[0] == "t":
                    emit_v_transpose(item[1])
                elif item[0] == "x":
                    emit_xt_load(item[1])
                elif item[0] == "a":
                    emit_proj_alloc(item[1])
                    emit_ot_alloc(item[1])

            fill_count = [0]

            def filler():
                fill_count[0] += 1
                if fill_count[0] % 3 != 0 or not work:
                    return
                do_item(work.pop(0))

            outproj_pending = []
            for b in range(B):
                for j in range(NQB):
                    emit_attention_qblk(b, j, filler)
                    outproj_pending.append((b, j))
                    if len(outproj_pending) > 1 or (b, j) == (B - 1, NQB - 1):
                        emit_outproj_qblk(*outproj_pending.pop(0))
            while work:   # safety: drain any unemitted deferred work
                do_item(work.pop(0))
            for pj in outproj_pending:
                emit_outproj_qblk(*pj)

    nc.compile()
    return nc


_NC_CACHE = {}


def _get_nc():
    if "nc" not in _NC_CACHE:
        _NC_CACHE["nc"] = build()
    return _NC_CACHE["nc"]


def make_in_maps(embeddings, Wq, bq, Wk, bk, Wv, bv, Wo, bo):
    embeddings = np.asarray(embeddings, dtype=np.float32)
    Wq, Wk, Wv = (np.asarray(a, np.float32) for a in (Wq, Wk, Wv))
    bq, bk, bv = (np.asarray(a, np.float32) for a in (bq, bk, bv))
    Wo = np.asarray(Wo, np.float32)

    xT = np.ascontiguousarray(embeddings.transpose(0, 2, 1)).astype(np.float16)
    e2 = np.zeros((33, 128), np.float16)
    e2[0, 0:64] = 1.0
    e2[32, 64:128] = 1.0

    in_maps = []
    for c in range(NCORES):
        h0, h1 = HPC * c, HPC * c + 1
        in_maps.append({
            "xT": xT,
            "wq2": np.concatenate([Wq[h0], Wq[h1]], axis=1).astype(np.float16),
            "wk2": np.concatenate([Wk[h0], Wk[h1]], axis=1).astype(np.float16),
            "wv2": np.concatenate([Wv[h0], Wv[h1]], axis=1).astype(np.float16),
            "bq2": np.concatenate([bq[h0], bq[h1]])[:, None].astype(np.float32),
            "bk2": np.concatenate([bk[h0], bk[h1]])[:, None].astype(np.float32),
            "bv2": np.concatenate([bv[h0], bv[h1]])[:, None].astype(np.float32),
            "wo2": Wo[HPC * DV * c: HPC * DV * (c + 1), :].astype(np.float16),
            "e2": e2,
        })
    return in_maps


def kernel(embeddings, Wq, bq, Wk, bk, Wv, bv, Wo, bo):
    nc = _get_nc()
    in_maps = make_in_maps(embeddings, Wq, bq, Wk, bk, Wv, bv, Wo, bo)
    res = run_bass_kernel_spmd(nc, in_maps, core_ids=list(range(NCORES)))
    acc = np.zeros((B * S, D), np.float32)
    for r in res.results:
        acc += r["out_t"].transpose(0, 2, 1, 3).reshape(B * S, D).astype(np.float32)
    acc += np.asarray(bo, np.float32)[None, :]
    return acc.reshape(B, S, D)
